# revision 54
# baseline (speedup 1.0000x reference)
"""Multi-head attention (B=8, S=1024, D=1024, H=16) on 8 TRN2 NeuronCores.

Sharding: pure data parallel — batch element b on core b. Weights are
broadcast to every core. No collectives.

v17: v12 + ramp/boundary/tail restructure + pair-end fill placement
(HW exec ~283-291us, mean ~285; run-to-run HAM/DMA phase puts +-4us on
identical code, so single runs within ~5us are noise; rel err 6.5e-3).
(fp8 DoubleRow was tried for the V path and halved its PE time, but the
e4m3 mantissa puts ~5e-2 relative noise on the attention output — the
softmax average shrinks signal and noise at the same sqrt(sum a^2) rate,
so nothing washes out; the Q/K path is even worse. The 2e-2 gate forces
bf16.)

  - ACT does ONLY exp (128 x [128,1024] = its ~141us floor). All
    PSUM->SBUF drains / bias-adds / casts run on DVE.
  - Normalization uses reciprocal_approx_fast (~5x cheaper than
    nc.vector.reciprocal). Its input must be copied out of PSUM first:
    the custom-DVE op reads garbage from PSUM operands. For the final
    pair the big pv copy is skipped (multiply reads PSUM directly) and
    the [1,512] denominator copies lead the chain.
  - Emission order respects dataflow (Tile derives dependencies from
    trace order!). X DMAs are emitted before everything (they gate the
    ramp); B(pr) is emitted one pair AHEAD of the D-loop consuming it so
    its qkT drains (DVE) land mid-phase — otherwise the pair-boundary
    normalize chain clogs DVE and strands the PE ~3.4us per pair.
  - sc=1 sweep fills (E chunks for s-tiles 0-3, EA partials) are
    emitted at sk 3/5/7 — late-in-pair placement lands them on the
    pair-end exp-wait bubbles and measured ~5us faster than sk 2/4/6.
  - Tail: E for s-tiles 4-7 is split dk0-4 (pre-accumulated into SBUF
    `ea` with bias folded in, interleaved into pairs 5-7 of the sc=1
    sweep) + dk5-7 finish chains after the last normalize; the final
    normalize window AND the finish-chain stretch (whose chains stall
    ~1-2.5us on y-tile/DVE pacing) are bridged by identity matmuls
    gated on the last exp tile (always-ready fillers migrate to the
    FIRST idle moment in the schedule, so they must carry a late
    dependency) and both 512-col finishes of an s-tile share one y
    tile and one out-DMA. ex bufs=5 (deeper ACT look-ahead) edged out
    4 in cool-device A/B.
  - Schedule is brittle: the priority-heap scheduler freezes one order
    per engine from a cost-model sim, and innocuous-looking emission or
    pool-size changes (e.g. staging W_v before vs after B(2), y bufs
    2->3) have swung the span by +-5-20us. Also measured as losses:
    PV lagging scores by 2 key tiles (moves the bubble to pair-start
    sps waits), folding the ea add into a PE identity-matmul with
    scalar.copy drains (ACT table switch), fewer ramp warmups, moving
    a sc=1 fill chunk from pair 3 to pair 0, and reordering the ramp's
    SP DMA-trigger queue (bias rows last) — the last one cost ~45us.
    Bench every change 2-3x; single runs within ~5us are noise.
  - The DEVICE also drifts: after ~2h of back-to-back benching the PE
    drops into the P0 power state (~2.0 GHz, spans ~335-344us for this
    exact kernel, a clean x1.20); ~4 min of idle restores 2.4 GHz and
    ~284-291us. Consecutive-run comparisons silently mix in this
    heating — prefer fresh-after-idle single runs when it matters.

Per-core algorithm (X: [S, D] for one batch element):
  1. X^T via PE transposes (bf16 matmul vs identity), cast to bf16.
  2. QK^T = W_in[:, :2D]^T @ X -> [2D, S] bf16, head-dim-on-partitions.
  3. V = X @ W_in[:, 2D:] natural, stored bf16 as V_aug[st, head, 65]
     with a ones column (col 64) so PV also produces the softmax
     denominator row.
  4. Per head pair: S^T = K_h^T.T @ Q_h^T (bf16, PE row-groups 0-63 /
     64-127 concurrently for the 2 heads), wide exp on ACT -> bf16
     (scale=1/8, max-subtraction skipped: scores ~N(0,1), exp <= e^7),
     PV [V_h | 1]^T @ exp accumulates out^T + denominator over sk.
  5. Normalize: reciprocal_approx_fast on row 64, GpSimd broadcast,
     DVE multiply -> attnT (bf16).
  6. Y = attn_out @ W_out + b_out in bf16.
"""

import sys

sys.path.insert(0, "/opt/trn_rl_repo")

import numpy as np

import concourse.bacc as bacc
import concourse.mybir as mybir
from concourse.bass_utils import run_bass_kernel_spmd
from concourse.masks import make_identity
from concourse.tile import TileContext

B = 8
S = 1024
D = 1024
H = 16
DK = D // H  # 64
P = 128
ST = S // P   # 8 s-tiles
DT = D // P   # 8 d-tiles
NTQK = 2 * D // P  # 16 n-tiles for the Q|K part
PAIRS = H // 2     # 8 head pairs
SC = S // 512      # 2 query chunks of 512 (matmul free-dim limit)

f32 = mybir.dt.float32
bf16 = mybir.dt.bfloat16
EXP = mybir.ActivationFunctionType.Exp
MULT = mybir.AluOpType.mult
ADD = mybir.AluOpType.add


def build_nc():
    nc = bacc.Bacc()
    X = nc.dram_tensor("X", [S, D], f32, kind="ExternalInput")
    W_in = nc.dram_tensor("W_in", [D, 3 * D], f32, kind="ExternalInput")
    b_in = nc.dram_tensor("b_in", [3 * D], f32, kind="ExternalInput")
    W_out = nc.dram_tensor("W_out", [D, D], f32, kind="ExternalInput")
    b_out = nc.dram_tensor("b_out", [D], f32, kind="ExternalInput")
    out = nc.dram_tensor("out", [S, D], f32, kind="ExternalOutput")

    w_in_kp = W_in.rearrange("(ko p) n -> p ko n", p=P)  # [128, 8, 3072]
    w_out_kp = W_out.rearrange("(ko p) n -> p ko n", p=P)  # [128, 8, 1024]

    with TileContext(nc) as tc:
        const = tc.alloc_tile_pool(name="const", bufs=1)
        # PSUM: sps 2x[128,1024] (4 banks) + pv 2x[128,512] (2 banks)
        #       + gp 2x[128,512] (2 banks) = 8 banks
        sps_pool = tc.alloc_tile_pool(name="spsp", bufs=2, space="PSUM")
        pv_pool = tc.alloc_tile_pool(name="pvp", bufs=1, space="PSUM")
        gp_pool = tc.alloc_tile_pool(name="gpp", bufs=2, space="PSUM")

        # X DMAs first: they gate the whole ramp (transposes -> QK -> scores)
        xs_pool = tc.alloc_tile_pool(name="xs", bufs=2)

        def phase_a_dma(st):
            x_tile = xs_pool.tile([P, D], f32, tag="x")
            nc.sync.dma_start(x_tile[:], X[st * P : (st + 1) * P, :])
            return x_tile

        x_stage = {st: phase_a_dma(st) for st in range(2)}

        identity = const.tile([P, P], bf16)
        make_identity(nc, identity[:])
        bqk = const.tile([P, NTQK], f32)
        nc.sync.dma_start(bqk[:], b_in[0 : 2 * D].rearrange("(o p) -> p o", p=P))
        bv_bc = const.tile([P, D], f32)
        bout_bc = const.tile([P, D], f32)
        ones_src = const.tile([P, ST * H], f32)
        nc.vector.memset(ones_src[:], 1.0)

        brow_pool = tc.alloc_tile_pool(name="brow", bufs=1)
        bv_row = brow_pool.tile([1, D], f32, tag="row")
        nc.sync.dma_start(bv_row[:], b_in[None, 2 * D : 3 * D])
        nc.gpsimd.partition_broadcast(bv_bc[:], bv_row[:])
        bout_row = brow_pool.tile([1, D], f32, tag="row")
        nc.sync.dma_start(bout_row[:], b_out[None, :])
        nc.gpsimd.partition_broadcast(bout_bc[:], bout_row[:])

        # ---------------- resident tensors ----------------
        xT_pool = tc.alloc_tile_pool(name="xT", bufs=1)
        xT = xT_pool.tile([P, DT, S], bf16)    # 2 MB
        qkT_pool = tc.alloc_tile_pool(name="qkT", bufs=1)
        qkT = qkT_pool.tile([P, NTQK, S], bf16)  # 4 MB
        vaug_pool = tc.alloc_tile_pool(name="vaug", bufs=1)
        v_aug = vaug_pool.tile([P, ST, H, DK + 1], bf16)  # 2.1 MB
        nc.vector.tensor_copy(
            v_aug[:, :, :, DK : DK + 1],
            ones_src[:].rearrange("p (s h one) -> p s h one", h=H, one=1),
        )
        attnT_pool = tc.alloc_tile_pool(name="attnT", bufs=1)
        attnT = attnT_pool.tile([P, DT, S], bf16)  # 2 MB
        wv_pool = tc.alloc_tile_pool(name="wv", bufs=1)
        wv = wv_pool.tile([P, DT, D], bf16)   # 2 MB
        wout_pool = tc.alloc_tile_pool(name="wout", bufs=1)
        wout = wout_pool.tile([P, DT, D], bf16)  # 2 MB

        # staging pools; big weights staged in 2-MB n-halves (SBUF budget)
        wvs_pool = tc.alloc_tile_pool(name="wvs", bufs=1)

        def stage_weight(dst, src_cols, ncx):
            wst = wvs_pool.tile([P, DT, 512], f32, tag="wstage")
            nc.sync.dma_start(wst[:], src_cols)
            for dg in range(2):
                nc.vector.tensor_copy(
                    dst[:, dg * 4 : (dg + 1) * 4, ncx * 512 : (ncx + 1) * 512],
                    wst[:, dg * 4 : (dg + 1) * 4, :],
                )

        xb_pool = tc.alloc_tile_pool(name="xb", bufs=2)
        wqs_pool = tc.alloc_tile_pool(name="wqs", bufs=2)
        wqb_pool = tc.alloc_tile_pool(name="wqb", bufs=2)
        ex_pool = tc.alloc_tile_pool(name="exp", bufs=5)
        nrm_pool = tc.alloc_tile_pool(name="nrm", bufs=1)
        y_pool = tc.alloc_tile_pool(name="yp", bufs=2)
        ea_pool = tc.alloc_tile_pool(name="ea", bufs=1)
        # E dk0-4 partials (+bias) for s-tiles 4-7; bf16 adds ~0.4% noise
        # on ~5/8 of Y — well inside the error budget, and f32 won't fit SBUF
        ea = ea_pool.tile([P, 8, 512], bf16)

        # ---------------- emission helpers ----------------
        def phase_a_compute(st):
            """cast + PE-transpose X s-tile st into xT."""
            x_tile = x_stage.pop(st)
            xb = xb_pool.tile([P, D], bf16, tag="xb")
            nc.vector.tensor_copy(xb[:], x_tile[:])
            for half in range(2):
                gp = gp_pool.tile([P, 512], f32, tag="gp", name="tpa")
                for j in range(4):
                    dj = half * 4 + j
                    nc.tensor.matmul(
                        gp[:, j * P : (j + 1) * P],
                        xb[:, dj * P : (dj + 1) * P],
                        identity[:],
                        start=True,
                        stop=True,
                    )
                sl = (slice(None), slice(half * 4, (half + 1) * 4),
                      slice(st * P, (st + 1) * P))
                nc.vector.tensor_copy(
                    xT[sl], gp[:].rearrange("p (j s) -> p j s", j=4)
                )

        def phase_c(st, ncxs=(0, 1)):
            """V projection -> v_aug[:, st, :, 0:64] (+bias)."""
            for ncx in ncxs:
                gp = gp_pool.tile([P, 512], f32, tag="gp", name="psc")
                for dk in range(DT):
                    nc.tensor.matmul(
                        gp[:],
                        xT[:, dk, st * P : (st + 1) * P],
                        wv[:, dk, ncx * 512 : (ncx + 1) * 512],
                        start=(dk == 0),
                        stop=(dk == DT - 1),
                    )
                hbase = ncx * (H // SC)
                nc.vector.tensor_tensor(
                    v_aug[:, st, hbase : hbase + H // SC, 0:DK],
                    gp[:].rearrange("p (h d) -> p h d", d=DK),
                    bv_bc[:, ncx * 512 : (ncx + 1) * 512].rearrange(
                        "p (h d) -> p h d", d=DK
                    ),
                    ADD,
                )

        def phase_b_half(nt, sc, w_tile, gp=None, dks=range(DT), drain=True):
            """QK^T projection chunk for n-tile nt, query chunk sc."""
            if gp is None:
                gp = gp_pool.tile([P, 512], f32, tag="gp", name="psb")
            for dk in dks:
                nc.tensor.matmul(
                    gp[:],
                    w_tile[:, dk, :],
                    xT[:, dk, sc * 512 : (sc + 1) * 512],
                    start=(dk == 0),
                    stop=(dk == DT - 1),
                )
            if drain:
                nc.vector.tensor_scalar(
                    qkT[:, nt, sc * 512 : (sc + 1) * 512],
                    gp[:],
                    bqk[:, nt : nt + 1],
                    None,
                    ADD,
                )
            return gp

        def phase_b_load_dma(nt):
            w_stage = wqs_pool.tile([P, DT, P], f32, tag="ws")
            nc.sync.dma_start(w_stage[:], w_in_kp[:, :, nt * P : (nt + 1) * P])
            return w_stage

        def phase_b_cast(w_stage):
            w_tile = wqb_pool.tile([P, DT, P], bf16, tag="w")
            nc.vector.tensor_copy(w_tile[:], w_stage[:])
            return w_tile

        def phase_b_load(nt):
            return phase_b_cast(phase_b_load_dma(nt))

        def phase_b(nt):
            w_tile = phase_b_load(nt)
            for sc in range(SC):
                phase_b_half(nt, sc, w_tile)

        def b_closures(nt_a, nt_b):
            """B(pair) as emission closures (8-MM halves)."""
            box = {}

            def load(which, nt):
                def f():
                    box[which] = phase_b_load(nt)
                return f

            def half(which, nt, sc):
                def f():
                    phase_b_half(nt, sc, box[which])
                return f

            return [
                load("a", nt_a), load("b", nt_b),
                half("a", nt_a, 0), half("b", nt_b, 0),
                half("a", nt_a, 1), half("b", nt_b, 1),
            ]

        def c_closures(st, ncx):
            def f():
                phase_c(st, ncxs=(ncx,))
            return [f]

        def e_closures(st):
            """Output projection s-tile as 2 closures (one per 512-col)."""
            def part(ncx):
                def f():
                    gp = gp_pool.tile([P, 512], f32, tag="gp", name="pse")
                    for dk in range(DT):
                        nc.tensor.matmul(
                            gp[:],
                            attnT[:, dk, st * P : (st + 1) * P],
                            wout[:, dk, ncx * 512 : (ncx + 1) * 512],
                            start=(dk == 0),
                            stop=(dk == DT - 1),
                        )
                    y = y_pool.tile([P, D], f32, tag="y")
                    nc.vector.tensor_tensor(
                        y[:, 0:512], gp[:],
                        bout_bc[:, ncx * 512 : (ncx + 1) * 512],
                        ADD,
                    )
                    nc.sync.dma_start(
                        out[st * P : (st + 1) * P,
                            ncx * 512 : (ncx + 1) * 512],
                        y[:, 0:512],
                    )
                return f
            return [part(0), part(1)]

        # E for s-tiles 4-7 (gated by the LAST pair's sc=1 normalize) is
        # split: dk0-4 partials (+bias) pre-accumulate into `ea` while
        # pairs 5-7 still run; the dk5-7 finish chains leave only one
        # pair-7-dependent matmul + drain per chunk for the tail.
        def e_partial(st, ncx):
            idx = (st - 4) * 2 + ncx
            gp = gp_pool.tile([P, 512], f32, tag="gp", name="psea")
            for dk in range(5):
                nc.tensor.matmul(
                    gp[:],
                    attnT[:, dk, st * P : (st + 1) * P],
                    wout[:, dk, ncx * 512 : (ncx + 1) * 512],
                    start=(dk == 0),
                    stop=(dk == 4),
                )
            nc.vector.tensor_tensor(
                ea[:, idx, :], gp[:], bout_bc[:, ncx * 512 : (ncx + 1) * 512],
                ADD,
            )

        def e_final2(st):
            """Both 512-col finish chains for s-tile st -> ONE out DMA.

            One dma_start per s-tile instead of two: the SP engine's
            descriptor generation (~0.8us per trigger) paces the tail
            otherwise. Drains alternate DVE / GpSimd so neither engine
            serializes the last chunks.
            """
            y = y_pool.tile([P, D], f32, tag="y")
            for ncx in range(SC):
                idx = (st - 4) * 2 + ncx
                gp = gp_pool.tile([P, 512], f32, tag="gp", name="psef")
                for dk in range(5, DT):
                    nc.tensor.matmul(
                        gp[:],
                        attnT[:, dk, st * P : (st + 1) * P],
                        wout[:, dk, ncx * 512 : (ncx + 1) * 512],
                        start=(dk == 5),
                        stop=(dk == DT - 1),
                    )
                nc.vector.tensor_tensor(
                    y[:, ncx * 512 : (ncx + 1) * 512], gp[:], ea[:, idx, :],
                    ADD,
                )
            nc.sync.dma_start(out[st * P : (st + 1) * P, :], y[:])

        def d_scores(pr, sc, sk):
            """Paired score matmuls for key tile sk."""
            sps = sps_pool.tile([P, S], f32, tag="sps", name="sps")
            for hh in range(2):
                base = hh * DK
                nc.tensor.matmul(
                    sps[:, hh * 512 : (hh + 1) * 512],
                    qkT[base : base + DK, PAIRS + pr, sk * P : (sk + 1) * P],
                    qkT[base : base + DK, pr, sc * 512 : (sc + 1) * 512],
                    start=True,
                    stop=True,
                )
            return sps

        def d_new_state():
            return {
                "pv": [
                    pv_pool.tile([P, 512], f32, tag=f"pv{i}", name=f"pv{i}")
                    for i in range(2)
                ],
                "ex": {},
            }

        def d_scores_exp(pr, sc, sk_range, state):
            """Scores + exp for key tiles in sk_range (no PV)."""
            for sk in sk_range:
                sps = d_scores(pr, sc, sk)
                ex = ex_pool.tile([P, S], bf16, tag="ex", name="ex")
                nc.scalar.activation(
                    ex[:], sps[:], EXP, scale=1.0 / np.sqrt(DK)
                )
                state["ex"][sk] = ex

        def d_pv(pr, sc, sk_range, state):
            """PV accumulation for key tiles in sk_range."""
            pv = state["pv"]
            for sk in sk_range:
                ex = state["ex"].pop(sk)
                for hh in range(2):
                    h = 2 * pr + hh
                    nc.tensor.matmul(
                        pv[hh][0 : DK + 1, :],
                        v_aug[:, sk, h, :],
                        ex[:, hh * 512 : (hh + 1) * 512],
                        start=(sk == 0),
                        stop=(sk == ST - 1),
                    )

        def phase_d(pr, sc):
            """Full attention pipeline for (pr, sc): scores/exp one key
            tile ahead of PV."""
            state = d_new_state()
            for sk in range(ST + 1):
                if sk < ST:
                    d_scores_exp(pr, sc, [sk], state)
                if sk >= 1:
                    d_pv(pr, sc, [sk - 1], state)
            return state

        def d_normalize(pr, sc, state, last=False):
            """attnT = pv * (1/denominator-row), denominator = pv row 64.

            Emission order minimizes chain latency: the [1,512] den copies
            (straight from PSUM -> SBUF; reciprocal_approx_fast needs an
            SBUF input — the custom-DVE op reads garbage from PSUM) go
            first so the recips + GpSimd broadcasts start early; the big
            pvc copies (which free the PSUM slot for the next pair's PV)
            follow; the multiplies last. For the final pair (last=True)
            there is no next pair, so the pvc copies are skipped entirely
            and the multiply reads the PSUM accumulator directly.
            """
            src = {}
            for hh in range(2):
                if last:
                    # no next pair -> no rush to free the PSUM bank; the
                    # multiply reads the accumulator directly and the den
                    # copy (straight from PSUM) leads the chain
                    src[hh] = state["pv"][hh]
                else:
                    # copies first: free the PSUM slot for the next pair's
                    # PV as early as possible
                    pvc = nrm_pool.tile(
                        [DK + 1, 512], f32, tag=f"pvc{hh}", name="pvc"
                    )
                    nc.vector.tensor_copy(pvc[:], state["pv"][hh][0 : DK + 1, :])
                    src[hh] = pvc
            den = {}
            rr = {}
            for hh in range(2):
                den[hh] = nrm_pool.tile([1, 512], f32, tag=f"dr{hh}", name="den")
                nc.vector.tensor_copy(den[hh][:], src[hh][DK : DK + 1, :])
            for hh in range(2):
                rr[hh] = nrm_pool.tile([1, 512], f32, tag=f"rr{hh}", name="rrow")
                nc.vector.reciprocal_approx_fast(rr[hh][:], den[hh][:])
            bcs = {}
            for hh in range(2):
                # full-tile broadcast (sliced outputs break on HW)
                bc = nrm_pool.tile([P, 512], f32, tag=f"bc{hh}", name="bc")
                nc.gpsimd.partition_broadcast(bc[:], rr[hh][:])
                bcs[hh] = bc
            for hh in range(2):
                base = hh * DK
                nc.vector.tensor_tensor(
                    attnT[base : base + DK, pr, sc * 512 : (sc + 1) * 512],
                    src[hh][0:DK, :],
                    bcs[hh][0:DK, :],
                    MULT,
                )

        def phase_e(st):
            """Output projection for s-tile st."""
            for ncx in range(SC):
                gp = gp_pool.tile([P, 512], f32, tag="gp", name="pse")
                for dk in range(DT):
                    nc.tensor.matmul(
                        gp[:],
                        attnT[:, dk, st * P : (st + 1) * P],
                        wout[:, dk, ncx * 512 : (ncx + 1) * 512],
                        start=(dk == 0),
                        stop=(dk == DT - 1),
                    )
                y = y_pool.tile([P, 512], f32, tag="y")
                nc.vector.tensor_tensor(
                    y[:], gp[:], bout_bc[:, ncx * 512 : (ncx + 1) * 512], ADD
                )
                nc.sync.dma_start(
                    out[st * P : (st + 1) * P, ncx * 512 : (ncx + 1) * 512],
                    y[:],
                )

        # ------------- emission order (must respect dataflow!) -------------
        # The Tile scheduler freezes ONE linear order per engine from a
        # cost-model simulation — no runtime gap-filling, and fine-grained
        # interleaving of unrelated matmul streams inflates LDWEIGHTS/sync
        # cost by ~25% (measured). So work stays in contiguous blocks, and
        # the blocks are ordered so no engine FIFO entry waits on late data.
        #
        # HAM: the PE clock starts throttled (1.2 GHz) and needs ~3.4us of
        # sustained activity to reach 2.4 GHz; identity matmuls at t=0
        # burn the X-DMA dead time to warm it up. X tiles 0-1 were DMA'd
        # at the very top; 2-3 and the pair-0 weight DMAs go out now so
        # the DMA queues stay ahead of the PE ramp.
        x_stage[2] = phase_a_dma(2)
        x_stage[3] = phase_a_dma(3)
        w0s = phase_b_load_dma(0)
        w8s = phase_b_load_dma(PAIRS)

        def keep_warm(n=4):
            gpw = gp_pool.tile([P, 512], f32, tag="gp", name="warm")
            for j in range(n):
                jj = j % 4
                nc.tensor.matmul(
                    gpw[:, jj * P : (jj + 1) * P],
                    identity[:], identity[:], start=True, stop=True,
                )

        for wu in range(20):
            keep_warm()

        for st in range(4):
            phase_a_compute(st)
            keep_warm(6)
        w0 = phase_b_cast(w0s)
        w8 = phase_b_cast(w8s)
        phase_b_half(0, 0, w0)
        phase_b_half(PAIRS, 0, w8)
        st0 = d_new_state()
        d_scores_exp(0, 0, range(0, 4), st0)
        for st in range(4, ST):
            x_stage[st] = phase_a_dma(st)
        for st in range(4, ST):
            phase_a_compute(st)
            keep_warm()
        phase_b_half(0, 1, w0)
        phase_b_half(PAIRS, 1, w8)
        d_scores_exp(0, 0, range(4, ST), st0)
        # pair-1's QK projection fills the PE while the W_v DMA lands
        # (also keeps HAM from re-throttling mid-ramp)
        w1 = phase_b_load(1)
        w9 = phase_b_load(PAIRS + 1)
        phase_b_half(1, 0, w1)
        phase_b_half(PAIRS + 1, 0, w9)
        phase_b_half(1, 1, w1)
        phase_b_half(PAIRS + 1, 1, w9)
        # W_v first half + the v_aug tiles pair 0 PV needs
        stage_weight(wv, w_in_kp[:, :, 2 * D : 2 * D + 512], 0)
        for st in range(4):
            phase_c(st, ncxs=(0,))
        d_pv(0, 0, range(0, 3), st0)
        for st in range(4, ST):
            phase_c(st, ncxs=(0,))
        d_pv(0, 0, range(3, ST), st0)
        d_normalize(0, 0, st0)
        # second V half (heads 8-15, pairs 4-7)
        stage_weight(wv, w_in_kp[:, :, 2 * D + 512 : 3 * D], 1)
        for st in range(ST):
            phase_c(st, ncxs=(1,))

        # B is emitted one pair AHEAD of the D loop that consumes it, so
        # each pair's qkT drains (DVE) land mid-phase — the pair-boundary
        # normalize chain otherwise clogs DVE and strands the PE for ~3.4us
        # waiting on the next pair's drains.
        phase_b(2)
        phase_b(PAIRS + 2)
        for pr in range(1, PAIRS):
            if 1 < pr < PAIRS - 1:
                phase_b(pr + 1)
                phase_b(PAIRS + pr + 1)
            state = d_new_state()
            for sk in range(ST + 1):
                if sk < ST:
                    d_scores_exp(pr, 0, [sk], state)
                if sk >= 1:
                    d_pv(pr, 0, [sk - 1], state)
            d_normalize(pr, 0, state)

        # W_out prefetch + cast (DMA during the sc=0 sweep)
        for ncx in range(SC):
            stage_weight(wout, w_out_kp[:, :, ncx * 512 : (ncx + 1) * 512], ncx)

        # sc=1 sweep. E(0..3) full chains interleave at pairs 2-5; the
        # E(4..7) dk0-4 partials interleave at pairs 5-7, with pair 7's two
        # partials held back until just before its normalize so they fill
        # the PE while the final normalize chain runs (their deps — pairs
        # 0-4 — are long done). The dk5-7 finish chains all come after the
        # last normalize: only ~5us of PE work gates the end of the kernel.
        # Fill work for the sc=1 sweep's ACT-paced stretches, one or two
        # ~1-2us chunks per pair so no single pair goes PE-heavy. E(st<4)
        # needs only sc=0 attnT (done); EA partials need pairs 0-4's sc=1.
        fill_sched = {
            0: [("e", 0, 0)],
            1: [("e", 0, 1), ("e", 1, 0)],
            2: [("e", 1, 1), ("e", 2, 0)],
            3: [("e", 2, 1), ("e", 3, 0)],
            4: [("e", 3, 1)],
            5: [("ea", 4, 0), ("ea", 4, 1), ("ea", 5, 0)],
            6: [("ea", 5, 1), ("ea", 6, 0), ("ea", 6, 1)],
        }
        for pr in range(PAIRS):
            state = d_new_state()
            ex_last = None
            fills = list(fill_sched.get(pr, []))
            for sk in range(ST + 1):
                if sk < ST:
                    d_scores_exp(pr, 1, [sk], state)
                    if sk == ST - 1:
                        ex_last = state["ex"][sk]
                if sk >= 1:
                    d_pv(pr, 1, [sk - 1], state)
                if fills and sk in (3, 5, 7):
                    kind, st, ncx = fills.pop(0)
                    if kind == "e":
                        e_closures(st)[ncx]()
                    else:
                        e_partial(st, ncx)
            if pr == PAIRS - 1:
                e_partial(7, 0)
                e_partial(7, 1)
                # PE filler for the final normalize window, gated on the
                # last exp so the scheduler can't migrate it earlier
                # (always-ready identity matmuls pop at the FIRST idle
                # moment anywhere in the kernel, not here where needed)
                for g in range(12):
                    gpw = gp_pool.tile([P, 512], f32, tag="gp", name="wfill")
                    nc.tensor.matmul(
                        gpw[:], identity[:],
                        ex_last[:, (g % 2) * 512 : (g % 2) * 512 + 512],
                        start=True, stop=True,
                    )
            d_normalize(pr, 1, state, last=(pr == PAIRS - 1))
            if pr == PAIRS - 1:
                ex_tail = ex_last
        for st in range(4, ST):
            e_final2(st)
            if st < ST - 1:
                # ex_last-gated filler: covers the e_final stretch's
                # y/DVE-pacing stalls and keeps HAM warm to the end
                for g in range(2):
                    gpw = gp_pool.tile([P, 512], f32, tag="gp", name="wfill2")
                    nc.tensor.matmul(
                        gpw[:], identity[:],
                        ex_tail[:, (g % 2) * 512 : (g % 2) * 512 + 512],
                        start=True, stop=True,
                    )

        for pool in (
            ea_pool, y_pool, nrm_pool, ex_pool, wqb_pool, wqs_pool, xb_pool,
            wvs_pool, wout_pool, wv_pool, attnT_pool, vaug_pool, qkT_pool,
            xT_pool, brow_pool, xs_pool, gp_pool, pv_pool, sps_pool, const,
        ):
            pool.release()

    nc.finalize()
    return nc


_NC_CACHE = {}


def get_nc():
    if "nc" not in _NC_CACHE:
        _NC_CACHE["nc"] = build_nc()
    return _NC_CACHE["nc"]


def kernel(X, W_in, b_in, W_out, b_out):
    X = np.ascontiguousarray(np.asarray(X, dtype=np.float32))
    W_in = np.ascontiguousarray(np.asarray(W_in, dtype=np.float32))
    b_in = np.ascontiguousarray(np.asarray(b_in, dtype=np.float32))
    W_out = np.ascontiguousarray(np.asarray(W_out, dtype=np.float32))
    b_out = np.ascontiguousarray(np.asarray(b_out, dtype=np.float32))

    nc = get_nc()
    in_maps = [
        {"X": X[i], "W_in": W_in, "b_in": b_in, "W_out": W_out, "b_out": b_out}
        for i in range(B)
    ]
    res = run_bass_kernel_spmd(nc, in_maps, core_ids=list(range(B)))
    return np.stack([res.results[i]["out"] for i in range(B)], axis=0)

